# revision 14
# baseline (speedup 1.0000x reference)
"""Trainium2 Bass kernel for a contextual loss (cosine-distance softmin loss).

Math (per batch b):
  mu_c      = mean_n Y[b,c,n]
  xc = X-mu, yc = Y-mu                      (centered, [C,N])
  psi[i,j]  = <xc_i, yc_j * ry_j>           (ry = 1/||yc_j||; f32r matmul)
  pm_i      = max_j psi[i,j]
  aa_i      = rx5_i / (1+EPS - 5*rx5_i*pm_i)    (rx5 = 0.2/||xc_i||)
  S_i       = sum_j exp(aa_i*(psi[i,j] - pm_i))
  CX_i      = 1/S_i ;  loss_b = -log(mean_i CX_i)

Sharding: 8 cores = 4 batches x 2 row-halves. Each core gets its
full-batch Y [64,4096] and its half of X's columns [64,2048], returns
S as [128,16] (partition p, chunk k <-> row k*128+p). Host reduces to
the [4] loss.

Layout: Y is loaded PACKED as [128,2048] - partitions 0-63 hold
channels for columns 0-2047, partitions 64-127 for columns 2048-4095.
All [*,N]-wide preamble work (squares, mean-subtract, ln/exp norms,
yhat) then uses all 128 lanes, and the K=64 main matmuls for the two
column halves run CONCURRENTLY in disjoint PE row-groups (rows 0-63 /
64-127), doubling effective PE throughput. Cross-half mixing (the mean
and the per-half channel sums) is done with two constant 0/1 matrices
(IdentMix / BlockOnes) fed as an extra input and used as f32r matmul
weights.

Per chunk: PE 8 f32r matmuls (issued h0/h1-interleaved) -> PSUM
[128,2048] x2; DVE TENSOR_MASK_REDUCE copies PSUM->SBUF rowbuf with a
running row-max; per PAIR of chunks a 6-op [128,2] DVE chain computes
aa/bb; ACT does ONE exp per chunk over [128,4096] with per-row
scale/bias and accumulated row-sum (the S output).
"""

import math

import numpy as np

import concourse.bacc as bacc
import concourse.mybir as mybir
from concourse.dve_ops import AFFINE_MUL_REDUCE, TENSOR_MASK_REDUCE
from concourse.bass_utils import run_bass_kernel_spmd
from concourse.mybir import ActivationFunctionType as AF, AluOpType as OP, AxisListType
from concourse.tile import TileContext

F32 = mybir.dt.float32
F32R = mybir.dt.float32r
BF16 = mybir.dt.bfloat16

B, C, N = 4, 64, 4096          # batch, channels, spatial (64*64)
NX = N // 2                    # rows per core (half batch)
CH = NX // 128                 # 16 chunks of 128 rows
HALF = N // 2                  # column half (one packed partition group)
H_BAND = 5.0
EPS_MIN = 1e-3
LN02 = math.log(0.2)

_NC_CACHE = {}


def build_nc():
    nc = bacc.Bacc("TRN2", target_bir_lowering=False, debug=False, num_devices=8)
    x_d = nc.dram_tensor("Xh", [C, NX], F32, kind="ExternalInput")
    y_d = nc.dram_tensor("Yb", [C, N], F32R, kind="ExternalInput")
    cst_d = nc.dram_tensor("Cst", [128, 256], F32R, kind="ExternalInput")
    out_d = nc.dram_tensor("out", [128, CH], F32, kind="ExternalOutput")

    with TileContext(nc) as tc:
        with (
            tc.tile_pool(name="persist", bufs=1) as persist,
            tc.tile_pool(name="mm", bufs=2, space="PSUM") as mmpool,
            tc.tile_pool(name="rb", bufs=4) as rbpool,
            tc.tile_pool(name="es", bufs=2) as espool,
            tc.tile_pool(name="small", bufs=4) as small,
        ):
            # ---------------- load inputs (packed Y, chunked for overlap) -----
            cst = persist.tile([128, 256], F32R)
            nc.sync.dma_start(out=cst[:], in_=cst_d[:])
            identmix = cst[:, 0:128]          # [k,p]=1 iff k%64==p%64
            blockones = cst[:, 128:256]       # [k,p]=1 iff k//64==p//64

            yp = persist.tile([128, HALF], F32R)   # packed Y
            for q in range(4):
                h, qq = divmod(q, 2)
                nc.sync.dma_start(
                    out=yp[64 * h:64 * (h + 1), qq * 1024:(qq + 1) * 1024],
                    in_=y_d[:, h * HALF + qq * 1024:h * HALF + (qq + 1) * 1024],
                )
            # X duplicated into both partition halves (for PE row-tiling)
            xdup = persist.tile([128, NX], F32)
            for h in range(2):
                nc.sync.dma_start(out=xdup[64 * h:64 * (h + 1), :], in_=x_d[:])

            ones_f = persist.tile([128, 64], F32)
            nc.vector.memset(ones_f[:], 1.0)
            ones1 = persist.tile([128, 2], F32R)
            nc.vector.tensor_copy(ones1[:], ones_f[:, 0:2])
            c3big = persist.tile([128, 1], F32)
            nc.vector.memset(c3big[:], 1.0e9)
            ln02 = persist.tile([128, 1], F32)
            nc.vector.memset(ln02[:], LN02)

            # squares of RAW packed Y overlap the DMA (no dependency on mu)
            ysq = persist.tile([128, HALF], F32R)
            for q in range(2):
                nc.scalar.activation(
                    ysq[:, q * 1024:(q + 1) * 1024],
                    yp[:, q * 1024:(q + 1) * 1024],
                    AF.Square,
                )

            # ---------------- mean of Y ---------------------------------------
            muparts = small.tile([128, 4], F32, tag="muparts")
            for q in range(4):
                nc.vector.reduce_sum(
                    out=muparts[:, q:q + 1],
                    in_=yp[:, q * 512:(q + 1) * 512].bitcast(F32),
                    axis=AxisListType.X,
                )
            musum = small.tile([128, 1], F32, tag="musum")
            nc.vector.reduce_sum(out=musum[:], in_=muparts[:], axis=AxisListType.X)
            musr = small.tile([128, 2], F32R, tag="musr")
            nc.vector.tensor_scalar(musr[:], ones_f[:, 0:2], musum[:], None, OP.mult)
            # full-row mean on every partition: mu[p] = sum of both halves / N
            ps_mu = mmpool.tile([128, HALF], F32, tag="mm")
            nc.tensor.matmul(
                ps_mu[:, 0:2], lhsT=identmix,
                rhs=musr[:], start=True, stop=True,
            )
            mu = small.tile([128, 1], F32, tag="mu")
            nc.vector.tensor_scalar_mul(mu[:], ps_mu[:, 0:1], 1.0 / N)
            muneg = small.tile([128, 1], F32, tag="muneg")
            nc.vector.tensor_scalar_mul(muneg[:], mu[:], -1.0)
            mu2neg = small.tile([128, 1], F32, tag="mu2neg")
            nc.vector.tensor_scalar_mul(mu2neg[:], mu[:], -2.0)

            # lhsT for the -2*mu@Y correction and for w = |mu|^2 broadcast
            lhs_mu = persist.tile([128, 128], F32R)
            nc.vector.tensor_scalar(
                lhs_mu[:], blockones.bitcast(F32), mu2neg[:], None, OP.mult
            )
            musq = small.tile([128, 1], F32, tag="musq")
            nc.vector.tensor_tensor(musq[:], mu[:], mu[:], OP.mult)
            lhs_w = persist.tile([128, 128], F32R)
            nc.vector.tensor_scalar(
                lhs_w[:], blockones.bitcast(F32), musq[:], None, OP.mult
            )
            ps_w = mmpool.tile([128, HALF], F32, tag="mm")
            nc.tensor.matmul(
                ps_w[:, 0:2], lhsT=lhs_w[:], rhs=ones1[:], start=True, stop=True
            )
            w_sb = small.tile([128, 1], F32, tag="w")
            nc.vector.tensor_copy(w_sb[:], ps_w[:, 0:1])

            # ---------------- X side: center, norms, rx5 ----------------------
            xcen = persist.tile([128, NX], F32R)
            nc.vector.tensor_scalar(xcen[:], xdup[:], mu[:], None, OP.subtract)
            xsq = persist.tile([C, NX], F32R)
            nc.scalar.activation(xsq[:], xcen[0:64, :], AF.Square)
            nx2 = mmpool.tile([128, HALF], F32, tag="mm")
            for k in range(CH):
                nc.tensor.matmul(
                    nx2[:, 2 * k:2 * k + 2],
                    lhsT=xsq[:, k * 128:(k + 1) * 128],
                    rhs=ones1[0:64, :],
                    start=True, stop=True,
                )
            tn = small.tile([128, CH], F32, tag="tn")
            nc.scalar.activation(
                tn[:], nx2[:, 0:2 * CH].rearrange("p (k two) -> p k two", two=2)[:, :, 0],
                AF.Ln,
            )
            # rx5 = 0.2/||xc||, qneg = -5*rx5 (so den = 1+eps - rx*pm)
            rx5 = persist.tile([128, CH], F32)
            nc.scalar.activation(rx5[:], tn[:], AF.Exp, bias=ln02[:], scale=-0.5)
            qneg = persist.tile([128, CH], F32)
            nc.vector.tensor_scalar_mul(qneg[:], rx5[:], -H_BAND)

            # ---------------- Y norms + yhat (all packed, one block) ----------
            # ny2 = BlockOnes@(Y^2) - 2mu@Y (accumulated), +w inside Ln's bias;
            # ry = exp(-0.5*ln(ny2+w)); yhat = (Y - mu)*ry in one fused op.
            ny2 = mmpool.tile([128, HALF], F32, tag="mm")
            for j in range(4):
                pj = slice(j * 512, (j + 1) * 512)
                nc.tensor.matmul(
                    ny2[:, pj], lhsT=blockones, rhs=ysq[:, pj],
                    start=True, stop=False,
                )
                nc.tensor.matmul(
                    ny2[:, pj], lhsT=lhs_mu[:], rhs=yp[:, pj],
                    start=False, stop=True,
                )
            yhat = persist.tile([128, HALF], F32R)
            for v in range(2):
                vs = slice(v * 1024, (v + 1) * 1024)
                tln = espool.tile([128, 1024], F32, tag="tln")
                nc.scalar.activation(tln[:], ny2[:, vs], AF.Ln, bias=w_sb[:])
                ry = rbpool.tile([128, 1024], F32, tag="ry")
                nc.scalar.activation(ry[:], tln[:], AF.Exp, scale=-0.5)
                nc.vector._custom_dve(
                    AFFINE_MUL_REDUCE,
                    out=yhat[:, vs],
                    in0=yp[:, vs].bitcast(F32),
                    in1=ry[:],
                    s0=1.0,
                    s1=muneg[:],
                )

            # ---------------- main loop ----------------
            pmall = persist.tile([128, CH], F32)
            aall = persist.tile([128, CH], F32)
            ball = persist.tile([128, CH], F32)
            ssums = persist.tile([128, CH], F32)
            rowbufs = {}
            for k in range(CH):
                ck = slice(k * 128, (k + 1) * 128)
                rowbuf = rbpool.tile([128, N], F32, tag="rb")
                rowbufs[k] = rowbuf
                ps0 = mmpool.tile([128, HALF], F32, tag="mm")
                ps1 = mmpool.tile([128, HALF], F32, tag="mm")
                ps = (ps0, ps1)
                for j in range(4):
                    pj = slice(j * 512, (j + 1) * 512)
                    for h in range(2):
                        hp = slice(64 * h, 64 * (h + 1))
                        nc.tensor.matmul(
                            ps[h][:, pj],
                            lhsT=xcen[hp, ck],
                            rhs=yhat[hp, pj],
                            start=True, stop=True,
                        )
                for h in range(2):
                    init = -3.0e38 if h == 0 else pmall[:, k:k + 1]
                    # rowbuf half = copy(ps); pmall[:,k] = max(rowmax, init)
                    nc.vector._custom_dve(
                        TENSOR_MASK_REDUCE,
                        out=rowbuf[:, h * HALF:(h + 1) * HALF],
                        in0=ps[h][:],
                        in1=c3big[:],
                        s0=0.0,
                        s1=init,
                        imm2=1.0,
                        accum_out=pmall[:, k:k + 1],
                    )

                if k % 2 == 1:
                    # per-row constants for chunks k-1,k as one [128,2] batch:
                    # aa = rx5/(1+eps - 5*rx5*pm), bb = -aa*pm
                    pr = slice(k - 1, k + 1)
                    t2 = small.tile([128, 2], F32, tag="t2")
                    nc.vector.tensor_tensor(t2[:], pmall[:, pr], qneg[:, pr], OP.mult)
                    den = small.tile([128, 2], F32, tag="den")
                    nc.vector.tensor_scalar(
                        den[:], t2[:], 1.0 + EPS_MIN, None, OP.add
                    )
                    rec = small.tile([128, 2], F32, tag="rec")
                    nc.vector.reciprocal_approx_fast(rec[:], den[:])
                    nc.vector.tensor_tensor(aall[:, pr], rec[:], rx5[:, pr], OP.mult)
                    t3 = small.tile([128, 2], F32, tag="t3")
                    nc.vector.tensor_tensor(t3[:], aall[:, pr], pmall[:, pr], OP.mult)
                    nc.vector.tensor_scalar_mul(ball[:, pr], t3[:], -1.0)

                    for kk in (k - 1, k):
                        es = espool.tile([128, N], BF16, tag="es")
                        nc.scalar.activation(
                            es[:],
                            rowbufs.pop(kk)[:],
                            AF.Exp,
                            scale=aall[:, kk:kk + 1],
                            bias=ball[:, kk:kk + 1],
                            accum_out=ssums[:, kk:kk + 1],
                        )

            nc.sync.dma_start(out=out_d[:], in_=ssums[:])

    nc.compile()
    return nc


def _get_nc():
    if "nc" not in _NC_CACHE:
        _NC_CACHE["nc"] = build_nc()
    return _NC_CACHE["nc"]


def _make_cst():
    k = np.arange(128)[:, None]
    p = np.arange(128)[None, :]
    identmix = (k % 64 == p % 64).astype(np.float32)
    blockones = (k // 64 == p // 64).astype(np.float32)
    return np.ascontiguousarray(np.concatenate([identmix, blockones], axis=1))


def make_in_maps(X_features, Y_features):
    X = np.ascontiguousarray(np.asarray(X_features, np.float32).reshape(B, C, N))
    Y = np.ascontiguousarray(np.asarray(Y_features, np.float32).reshape(B, C, N))
    cst = _make_cst()
    in_maps = []
    for c in range(8):
        b, h = divmod(c, 2)
        in_maps.append({
            "Xh": np.ascontiguousarray(X[b, :, h * NX:(h + 1) * NX]),
            "Yb": Y[b],
            "Cst": cst,
        })
    return in_maps


def combine(results):
    """results: list of 8 dicts with 'out' [128, CH] = S per row."""
    out = np.empty(B, np.float32)
    for b in range(B):
        tot = 0.0
        for h in range(2):
            s = results[2 * b + h]["out"].astype(np.float64)
            tot += (1.0 / s).sum()
        out[b] = -np.log(tot / N)
    return out


def kernel(X_features, Y_features):
    nc = _get_nc()
    in_maps = make_in_maps(X_features, Y_features)
    res = run_bass_kernel_spmd(nc, in_maps, core_ids=list(range(8)))
    return combine(res.results)


if __name__ == "__main__":
    rng = np.random.default_rng(0)
    X = rng.standard_normal((B, C, 64, 64)).astype(np.float32)
    Y = rng.standard_normal((B, C, 64, 64)).astype(np.float32)
    print(kernel(X_features=X, Y_features=Y))


# revision 19
# speedup vs baseline: 1.1167x; 1.1167x over previous
"""Trainium2 Bass kernel for a contextual loss (cosine-distance softmin loss).

Math (per batch b):
  mu_c      = mean_n Y[b,c,n]
  xc = X-mu, yc = Y-mu                      (centered, [C,N])
  psi[i,j]  = <xc_i, yc_j * ry_j>           (ry = 1/||yc_j||; f32r matmul)
  pm_i      = max_j psi[i,j]
  aa_i      = rx5_i / (1+EPS - 5*rx5_i*pm_i)    (rx5 = 0.2/||xc_i||)
  S_i       = sum_j exp(aa_i*(psi[i,j] - pm_i))
  CX_i      = 1/S_i ;  loss_b = -log(mean_i CX_i)

Sharding: 8 cores = 4 batches x 2 row-halves. Each core gets its
full-batch Y [64,4096] and its half of X's columns [64,2048], returns
S as [128,16] (partition p, chunk k <-> row k*128+p). Host reduces to
the [4] loss.

Layout: Y is loaded PACKED as [128,2048] - partitions 0-63 hold
channels for columns 0-2047, partitions 64-127 for columns 2048-4095,
so all [*,N]-wide preamble work uses all 128 lanes and the K=64 main
matmuls for the two column halves target disjoint PE row-groups.
Cross-half mixing (the mean and per-half channel sums) uses two
constant 0/1 matrices (IdentMix / BlockOnes) fed as an extra input.

Engine budget: the DVE is the bottleneck (32 PSUM->SBUF mask-reduce
copies with fused running row-max). ACT runs only Ln/Exp (squares are
DVE tensor_tensor ops) so a single activation table load suffices.
The exp is ONE ACT op per chunk over [128,4096] with per-row
scale/bias and an accumulated row-sum; per-row constants aa/bb are
computed for PAIRS of chunks ([128,2] ops) to halve small-op count.
"""

import math

import numpy as np

import concourse.bacc as bacc
import concourse.mybir as mybir
from concourse.dve_ops import AFFINE_MUL_REDUCE, TENSOR_MASK_REDUCE
from concourse.bass_utils import run_bass_kernel_spmd
from concourse.mybir import ActivationFunctionType as AF, AluOpType as OP, AxisListType
from concourse.tile import TileContext

F32 = mybir.dt.float32
F32R = mybir.dt.float32r
BF16 = mybir.dt.bfloat16

B, C, N = 4, 64, 4096          # batch, channels, spatial (64*64)
NX = N // 2                    # rows per core (half batch)
CH = NX // 128                 # 16 chunks of 128 rows
HALF = N // 2                  # column half (one packed partition group)
H_BAND = 5.0
EPS_MIN = 1e-3
LN02 = math.log(0.2)

_NC_CACHE = {}


def build_nc():
    nc = bacc.Bacc("TRN2", target_bir_lowering=False, debug=False, num_devices=8)
    x_d = nc.dram_tensor("Xh", [C, NX], F32, kind="ExternalInput")
    y_d = nc.dram_tensor("Yb", [C, N], F32R, kind="ExternalInput")
    cst_d = nc.dram_tensor("Cst", [128, 256], F32R, kind="ExternalInput")
    out_d = nc.dram_tensor("out", [128, CH], F32, kind="ExternalOutput")

    with TileContext(nc) as tc:
        with (
            tc.tile_pool(name="persist", bufs=1) as persist,
            tc.tile_pool(name="mm", bufs=2, space="PSUM") as mmpool,
            tc.tile_pool(name="rb", bufs=6) as rbpool,
            tc.tile_pool(name="ry", bufs=2) as rypool,
            tc.tile_pool(name="es", bufs=2) as espool,
            tc.tile_pool(name="small", bufs=4) as small,
        ):
            # ---------------- load inputs (packed Y first, then X) ------------
            cst = persist.tile([128, 256], F32R)
            nc.sync.dma_start(out=cst[:], in_=cst_d[:])
            identmix = cst[:, 0:128]          # [k,p]=1 iff k%64==p%64
            blockones = cst[:, 128:256]       # [k,p]=1 iff k//64==p//64

            yp = persist.tile([128, HALF], F32R)   # packed Y
            for q in range(8):
                h, qq = divmod(q, 4)
                nc.sync.dma_start(
                    out=yp[64 * h:64 * (h + 1), qq * 512:(qq + 1) * 512],
                    in_=y_d[:, h * HALF + qq * 512:h * HALF + (qq + 1) * 512],
                )
            # X duplicated into both partition halves (for PE row-tiling)
            xdup = persist.tile([128, NX], F32)
            for h in range(2):
                nc.sync.dma_start(out=xdup[64 * h:64 * (h + 1), :], in_=x_d[:])

            ones_f = persist.tile([128, 64], F32)
            nc.vector.memset(ones_f[:], 1.0)
            ones1 = persist.tile([128, 2], F32R)
            nc.vector.tensor_copy(ones1[:], ones_f[:, 0:2])
            c3big = persist.tile([128, 1], F32)
            nc.vector.memset(c3big[:], 1.0e9)
            ln02 = persist.tile([128, 1], F32)
            nc.vector.memset(ln02[:], LN02)

            # one PSUM tile shared by all small preamble matmuls (disjoint
            # column ranges), so the 2-buffer PSUM pool never deadlocks on
            # the long-lived ny2 tile.
            aux = mmpool.tile([128, HALF], F32, tag="mm")
            ps_mu = aux[:, 0:2]
            ps_w = aux[:, 2:4]
            nx2 = aux[:, 64:64 + 2 * CH]

            # squares + mean partials of RAW packed Y overlap the DMA
            ysq = persist.tile([128, HALF], F32R)
            muparts = small.tile([128, 4], F32, tag="muparts")
            ny2 = mmpool.tile([128, HALF], F32, tag="mm")
            for q in range(4):
                qs = slice(q * 512, (q + 1) * 512)
                ypq = yp[:, qs].bitcast(F32)
                nc.vector.tensor_tensor(ysq[:, qs], ypq, ypq, OP.mult)
                nc.vector.reduce_sum(
                    out=muparts[:, q:q + 1], in_=ypq, axis=AxisListType.X
                )
                # ny2 partial: BlockOnes @ Y^2  (finished by the -2mu@Y MM below)
                nc.tensor.matmul(
                    ny2[:, qs], lhsT=blockones, rhs=ysq[:, qs],
                    start=True, stop=False,
                )

            musum = small.tile([128, 1], F32, tag="musum")
            nc.vector.reduce_sum(
                out=musum[:], in_=muparts[:], axis=AxisListType.X
            )
            musr = small.tile([128, 2], F32R, tag="musr")
            nc.vector.tensor_scalar(musr[:], ones_f[:, 0:2], musum[:], None, OP.mult)
            # full-row mean on every partition: mu[p] = sum of both halves / N
            nc.tensor.matmul(
                ps_mu, lhsT=identmix, rhs=musr[:], start=True, stop=True
            )
            mu = small.tile([128, 1], F32, tag="mu")
            nc.vector.tensor_scalar_mul(mu[:], ps_mu[:, 0:1], 1.0 / N)
            muneg = small.tile([128, 1], F32, tag="muneg")
            nc.vector.tensor_scalar_mul(muneg[:], mu[:], -1.0)
            mu2neg = small.tile([128, 1], F32, tag="mu2neg")
            nc.vector.tensor_scalar_mul(mu2neg[:], mu[:], -2.0)

            # lhsT for the -2*mu@Y correction and for w = |mu|^2 broadcast
            lhs_mu = persist.tile([128, 128], F32R)
            nc.vector.tensor_scalar(
                lhs_mu[:], blockones.bitcast(F32), mu2neg[:], None, OP.mult
            )
            musq = small.tile([128, 1], F32, tag="musq")
            nc.vector.tensor_tensor(musq[:], mu[:], mu[:], OP.mult)
            lhs_w = persist.tile([128, 128], F32R)
            nc.vector.tensor_scalar(
                lhs_w[:], blockones.bitcast(F32), musq[:], None, OP.mult
            )
            nc.tensor.matmul(
                ps_w, lhsT=lhs_w[:], rhs=ones1[:], start=True, stop=True
            )
            w_sb = small.tile([128, 1], F32, tag="w")
            nc.vector.tensor_copy(w_sb[:], ps_w[:, 0:1])

            # ---------------- Y norms + yhat (packed halves) ------------------
            # finish ny2 with the -2mu@Y MMs; ry = exp(-0.5*ln(ny2+w));
            # yhat = (Y - mu)*ry in one fused DVE op.
            yhat = persist.tile([128, HALF], F32R)
            for v in range(2):
                vs = slice(v * 1024, (v + 1) * 1024)
                for j in range(2):
                    pj = slice(v * 1024 + j * 512, v * 1024 + (j + 1) * 512)
                    nc.tensor.matmul(
                        ny2[:, pj], lhsT=lhs_mu[:], rhs=yp[:, pj],
                        start=False, stop=True,
                    )
                tln = rypool.tile([128, 1024], F32, tag="tln")
                nc.scalar.activation(tln[:], ny2[:, vs], AF.Ln, bias=w_sb[:])
                ry = rypool.tile([128, 1024], F32, tag="ry")
                nc.scalar.activation(ry[:], tln[:], AF.Exp, scale=-0.5)
                nc.vector._custom_dve(
                    AFFINE_MUL_REDUCE,
                    out=yhat[:, vs],
                    in0=yp[:, vs].bitcast(F32),
                    in1=ry[:],
                    s0=1.0,
                    s1=muneg[:],
                )

            # ---------------- X side: center, norms, rx5 ----------------------
            xcen = persist.tile([128, NX], F32R)
            nc.vector.tensor_scalar(xcen[:], xdup[:], mu[:], None, OP.subtract)
            xsq = persist.tile([C, NX], F32R)
            nc.vector.tensor_tensor(
                xsq[:], xcen[0:64, :].bitcast(F32), xcen[0:64, :].bitcast(F32),
                OP.mult,
            )
            for k in range(CH):
                nc.tensor.matmul(
                    nx2[:, 2 * k:2 * k + 2],
                    lhsT=xsq[:, k * 128:(k + 1) * 128],
                    rhs=ones1[0:64, :],
                    start=True, stop=True,
                )
            tn = small.tile([128, CH], F32, tag="tn")
            nc.scalar.activation(
                tn[:], nx2.rearrange("p (k two) -> p k two", two=2)[:, :, 0],
                AF.Ln,
            )
            # rx5 = 0.2/||xc||, qneg = -5*rx5 (so den = 1+eps - rx*pm)
            rx5 = persist.tile([128, CH], F32)
            nc.scalar.activation(rx5[:], tn[:], AF.Exp, bias=ln02[:], scale=-0.5)
            qneg = persist.tile([128, CH], F32)
            nc.vector.tensor_scalar_mul(qneg[:], rx5[:], -H_BAND)

            # ---------------- main loop ----------------
            pmall = persist.tile([128, CH], F32)
            aall = persist.tile([128, CH], F32)
            ball = persist.tile([128, CH], F32)
            ssums = persist.tile([128, CH], F32)
            rowbufs = {}

            def chain_and_exp(ks):
                # aa = rx5/(1+eps - 5*rx5*pm), bb = -aa*pm for chunks ks
                pr = slice(ks[0], ks[-1] + 1)
                nn = len(ks)
                t2 = small.tile([128, nn], F32, tag="t2")
                nc.vector.tensor_tensor(t2[:], pmall[:, pr], qneg[:, pr], OP.mult)
                den = small.tile([128, nn], F32, tag="den")
                nc.vector.tensor_scalar(den[:], t2[:], 1.0 + EPS_MIN, None, OP.add)
                rec = small.tile([128, nn], F32, tag="rec")
                nc.vector.reciprocal_approx_fast(rec[:], den[:])
                nc.vector.tensor_tensor(aall[:, pr], rec[:], rx5[:, pr], OP.mult)
                t3 = small.tile([128, nn], F32, tag="t3")
                nc.vector.tensor_tensor(t3[:], aall[:, pr], pmall[:, pr], OP.mult)
                nc.vector.tensor_scalar_mul(ball[:, pr], t3[:], -1.0)
                for kk in ks:
                    es = espool.tile([128, N], BF16, tag="es")
                    nc.scalar.activation(
                        es[:],
                        rowbufs.pop(kk)[:],
                        AF.Exp,
                        scale=aall[:, kk:kk + 1],
                        bias=ball[:, kk:kk + 1],
                        accum_out=ssums[:, kk:kk + 1],
                    )

            for k in range(CH):
                ck = slice(k * 128, (k + 1) * 128)
                rowbuf = rbpool.tile([128, N], F32, tag="rb")
                rowbufs[k] = rowbuf
                ps0 = mmpool.tile([128, HALF], F32, tag="mm")
                ps1 = mmpool.tile([128, HALF], F32, tag="mm")
                ps = (ps0, ps1)
                for j in range(4):
                    pj = slice(j * 512, (j + 1) * 512)
                    for h in range(2):
                        hp = slice(64 * h, 64 * (h + 1))
                        nc.tensor.matmul(
                            ps[h][:, pj],
                            lhsT=xcen[hp, ck],
                            rhs=yhat[hp, pj],
                            start=True, stop=True,
                        )
                for h in range(2):
                    init = -3.0e38 if h == 0 else pmall[:, k:k + 1]
                    # rowbuf half = copy(ps); pmall[:,k] = max(rowmax, init)
                    nc.vector._custom_dve(
                        TENSOR_MASK_REDUCE,
                        out=rowbuf[:, h * HALF:(h + 1) * HALF],
                        in0=ps[h][:],
                        in1=c3big[:],
                        s0=0.0,
                        s1=init,
                        imm2=1.0,
                        accum_out=pmall[:, k:k + 1],
                    )

                if k >= CH - 2:
                    chain_and_exp((k,))        # shorter tail for last chunks
                elif k % 2 == 1:
                    chain_and_exp((k - 1, k))

            nc.sync.dma_start(out=out_d[:], in_=ssums[:])

    nc.compile()
    return nc


def _get_nc():
    if "nc" not in _NC_CACHE:
        _NC_CACHE["nc"] = build_nc()
    return _NC_CACHE["nc"]


def _make_cst():
    k = np.arange(128)[:, None]
    p = np.arange(128)[None, :]
    identmix = (k % 64 == p % 64).astype(np.float32)
    blockones = (k // 64 == p // 64).astype(np.float32)
    return np.ascontiguousarray(np.concatenate([identmix, blockones], axis=1))


def make_in_maps(X_features, Y_features):
    X = np.ascontiguousarray(np.asarray(X_features, np.float32).reshape(B, C, N))
    Y = np.ascontiguousarray(np.asarray(Y_features, np.float32).reshape(B, C, N))
    cst = _make_cst()
    in_maps = []
    for c in range(8):
        b, h = divmod(c, 2)
        in_maps.append({
            "Xh": np.ascontiguousarray(X[b, :, h * NX:(h + 1) * NX]),
            "Yb": Y[b],
            "Cst": cst,
        })
    return in_maps


def combine(results):
    """results: list of 8 dicts with 'out' [128, CH] = S per row."""
    out = np.empty(B, np.float32)
    for b in range(B):
        tot = 0.0
        for h in range(2):
            s = results[2 * b + h]["out"].astype(np.float64)
            tot += (1.0 / s).sum()
        out[b] = -np.log(tot / N)
    return out


def kernel(X_features, Y_features):
    nc = _get_nc()
    in_maps = make_in_maps(X_features, Y_features)
    res = run_bass_kernel_spmd(nc, in_maps, core_ids=list(range(8)))
    return combine(res.results)


if __name__ == "__main__":
    rng = np.random.default_rng(0)
    X = rng.standard_normal((B, C, 64, 64)).astype(np.float32)
    Y = rng.standard_normal((B, C, 64, 64)).astype(np.float32)
    print(kernel(X_features=X, Y_features=Y))


# revision 23
# speedup vs baseline: 1.1407x; 1.0215x over previous
"""Trainium2 Bass kernel for a contextual loss (cosine-distance softmin loss).

Math (per batch b):
  mu_c      = mean_n Y[b,c,n]
  xc = X-mu, yc = Y-mu                      (centered, [C,N])
  psi[i,j]  = <xc_i, yc_j * ry_j>           (ry = 1/||yc_j||; f32r matmul)
  pm_i      = max_j psi[i,j]
  aa_i      = rx5_i / (1+EPS - 5*rx5_i*pm_i)    (rx5 = 0.2/||xc_i||)
  S_i       = sum_j exp(aa_i*(psi[i,j] - pm_i))
  CX_i      = 1/S_i ;  loss_b = -log(mean_i CX_i)

Sharding: 8 cores = 4 batches x 2 row-halves. Each core gets its
full-batch Y [64,4096] and its half of X's columns [64,2048], returns
S as [128,16] (partition p, chunk k <-> row k*128+p). Host reduces to
the [4] loss.

Layout: Y is loaded PACKED as [128,2048] - partitions 0-63 hold
channels for columns 0-2047, partitions 64-127 for columns 2048-4095,
so all [*,N]-wide preamble work uses all 128 lanes and the K=64 main
matmuls for the two column halves target disjoint PE row-groups.
Cross-half mixing (the mean and per-half channel sums) uses two
constant 0/1 matrices (IdentMix / BlockOnes) fed as an extra input.

Engine budget: the DVE is the bottleneck (32 PSUM->SBUF mask-reduce
copies with fused running row-max). ACT runs only Ln/Exp (squares are
DVE tensor_tensor ops) so a single activation table load suffices.
The exp is ONE ACT op per chunk over [128,4096] with per-row
scale/bias and an accumulated row-sum; per-row constants aa/bb are
computed for PAIRS of chunks ([128,2] ops) to halve small-op count.
"""

import math

import numpy as np

import concourse.bacc as bacc
import concourse.mybir as mybir
from concourse.dve_ops import AFFINE_MUL_REDUCE, TENSOR_MASK_REDUCE
from concourse.bass_utils import run_bass_kernel_spmd
from concourse.mybir import ActivationFunctionType as AF, AluOpType as OP, AxisListType
from concourse.tile import TileContext

F32 = mybir.dt.float32
F32R = mybir.dt.float32r
BF16 = mybir.dt.bfloat16

B, C, N = 4, 64, 4096          # batch, channels, spatial (64*64)
NX = N // 2                    # rows per core (half batch)
CH = NX // 128                 # 16 chunks of 128 rows
HALF = N // 2                  # column half (one packed partition group)
H_BAND = 5.0
EPS_MIN = 1e-3
LN02 = math.log(0.2)

_NC_CACHE = {}


def build_nc():
    nc = bacc.Bacc("TRN2", target_bir_lowering=False, debug=False, num_devices=8)
    x_d = nc.dram_tensor("Xh", [C, NX], F32, kind="ExternalInput")
    y_d = nc.dram_tensor("Yb", [C, N], F32R, kind="ExternalInput")
    cst_d = nc.dram_tensor("Cst", [128, 256], F32R, kind="ExternalInput")
    out_d = nc.dram_tensor("out", [128, CH], F32, kind="ExternalOutput")

    with TileContext(nc) as tc:
        with (
            tc.tile_pool(name="persist", bufs=1) as persist,
            tc.tile_pool(name="mm", bufs=2, space="PSUM") as mmpool,
            tc.tile_pool(name="rb", bufs=6) as rbpool,
            tc.tile_pool(name="ry", bufs=2) as rypool,
            tc.tile_pool(name="es", bufs=2) as espool,
            tc.tile_pool(name="small", bufs=4) as small,
        ):
            # ---------------- load inputs (packed Y first, then X) ------------
            cst = persist.tile([128, 256], F32R)
            nc.sync.dma_start(out=cst[:], in_=cst_d[:])
            identmix = cst[:, 0:128]          # [k,p]=1 iff k%64==p%64
            blockones = cst[:, 128:256]       # [k,p]=1 iff k//64==p//64

            yp = persist.tile([128, HALF], F32R)   # packed Y
            for q in range(4):
                h, qq = divmod(q, 2)
                nc.sync.dma_start(
                    out=yp[64 * h:64 * (h + 1), qq * 1024:(qq + 1) * 1024],
                    in_=y_d[:, h * HALF + qq * 1024:h * HALF + (qq + 1) * 1024],
                )
            x_sb = persist.tile([C, NX], F32)
            nc.sync.dma_start(out=x_sb[:], in_=x_d[:])

            ones_f = persist.tile([128, 64], F32)
            nc.vector.memset(ones_f[:], 1.0)
            ones1 = persist.tile([128, 2], F32R)
            nc.vector.tensor_copy(ones1[:], ones_f[:, 0:2])
            c3big = persist.tile([128, 1], F32)
            nc.vector.memset(c3big[:], 1.0e9)
            ln02 = persist.tile([128, 1], F32)
            nc.vector.memset(ln02[:], LN02)

            # one PSUM tile shared by all small preamble matmuls (disjoint
            # column ranges), so the 2-buffer PSUM pool never deadlocks on
            # the long-lived ny2 tile.
            aux = mmpool.tile([128, HALF], F32, tag="mm")
            ps_mu = aux[:, 0:2]
            ps_w = aux[:, 2:4]
            nx2 = aux[:, 64:64 + 2 * CH]

            # squares + mean partials of RAW packed Y overlap the DMA
            ysq = persist.tile([128, HALF], F32R)
            muparts = small.tile([128, 2], F32, tag="muparts")
            ny2 = mmpool.tile([128, HALF], F32, tag="mm")
            for q in range(2):
                qs = slice(q * 1024, (q + 1) * 1024)
                ypq = yp[:, qs].bitcast(F32)
                nc.vector.reduce_sum(
                    out=muparts[:, q:q + 1], in_=ypq, axis=AxisListType.X
                )
                nc.vector.tensor_tensor(ysq[:, qs], ypq, ypq, OP.mult)
                # ny2 partial: BlockOnes @ Y^2  (finished by the -2mu@Y MM below)
                for j in range(2):
                    pj = slice(q * 1024 + j * 512, q * 1024 + (j + 1) * 512)
                    nc.tensor.matmul(
                        ny2[:, pj], lhsT=blockones, rhs=ysq[:, pj],
                        start=True, stop=False,
                    )

            musum = small.tile([128, 1], F32, tag="musum")
            nc.vector.reduce_sum(
                out=musum[:], in_=muparts[:], axis=AxisListType.X
            )
            musr = small.tile([128, 2], F32R, tag="musr")
            nc.vector.tensor_scalar(musr[:], ones_f[:, 0:2], musum[:], None, OP.mult)
            # full-row mean on every partition: mu[p] = sum of both halves / N
            nc.tensor.matmul(
                ps_mu, lhsT=identmix, rhs=musr[:], start=True, stop=True
            )
            mu = small.tile([128, 1], F32, tag="mu")
            nc.vector.tensor_scalar_mul(mu[:], ps_mu[:, 0:1], 1.0 / N)
            muneg = small.tile([128, 1], F32, tag="muneg")
            nc.vector.tensor_scalar_mul(muneg[:], mu[:], -1.0)
            mu2neg = small.tile([128, 1], F32, tag="mu2neg")
            nc.vector.tensor_scalar_mul(mu2neg[:], mu[:], -2.0)

            # lhsT for the -2*mu@Y correction and for w = |mu|^2 broadcast
            lhs_mu = persist.tile([128, 128], F32R)
            nc.vector.tensor_scalar(
                lhs_mu[:], blockones.bitcast(F32), mu2neg[:], None, OP.mult
            )
            musq = small.tile([128, 1], F32, tag="musq")
            nc.vector.tensor_tensor(musq[:], mu[:], mu[:], OP.mult)
            lhs_w = persist.tile([128, 128], F32R)
            nc.vector.tensor_scalar(
                lhs_w[:], blockones.bitcast(F32), musq[:], None, OP.mult
            )
            nc.tensor.matmul(
                ps_w, lhsT=lhs_w[:], rhs=ones1[:], start=True, stop=True
            )
            w_sb = small.tile([128, 1], F32, tag="w")
            nc.vector.tensor_copy(w_sb[:], ps_w[:, 0:1])

            # ---------------- Y norms + yhat (packed halves) ------------------
            # finish ny2 with the -2mu@Y MMs; ry = exp(-0.5*ln(ny2+w));
            # yhat = (Y - mu)*ry in one fused DVE op. Both Lns are emitted
            # before both Exps so the activation table set switches ONCE.
            yhat = persist.tile([128, HALF], F32R)
            for j in range(4):
                pj = slice(j * 512, (j + 1) * 512)
                nc.tensor.matmul(
                    ny2[:, pj], lhsT=lhs_mu[:], rhs=yp[:, pj],
                    start=False, stop=True,
                )
            tln0 = rypool.tile([128, 1024], F32, tag="tln")
            nc.scalar.activation(tln0[:], ny2[:, 0:1024], AF.Ln, bias=w_sb[:])
            tln1 = rypool.tile([128, 1024], F32, tag="tln")
            nc.scalar.activation(tln1[:], ny2[:, 1024:2048], AF.Ln, bias=w_sb[:])
            for v, tln in ((0, tln0), (1, tln1)):
                vs = slice(v * 1024, (v + 1) * 1024)
                ry = rypool.tile([128, 1024], F32, tag="ry")
                nc.scalar.activation(ry[:], tln[:], AF.Exp, scale=-0.5)
                nc.vector._custom_dve(
                    AFFINE_MUL_REDUCE,
                    out=yhat[:, vs],
                    in0=yp[:, vs].bitcast(F32),
                    in1=ry[:],
                    s0=1.0,
                    s1=muneg[:],
                )

            # ---------------- X side: center, norms, rx5 ----------------------
            # X is centered on partitions 0-63 and duplicated to 64-127 by an
            # SBUF->SBUF DMA (PE row-tiling needs both row-groups).
            xcen = persist.tile([128, NX], F32R)
            nc.vector.tensor_scalar(xcen[0:64, :], x_sb[:], mu[0:64, :], None, OP.subtract)
            nc.sync.dma_start(out=xcen[64:128, :], in_=xcen[0:64, :])
            xsq = persist.tile([C, NX], F32R)
            nc.vector.tensor_tensor(
                xsq[:], xcen[0:64, :].bitcast(F32), xcen[0:64, :].bitcast(F32),
                OP.mult,
            )
            for k in range(CH):
                nc.tensor.matmul(
                    nx2[:, 2 * k:2 * k + 2],
                    lhsT=xsq[:, k * 128:(k + 1) * 128],
                    rhs=ones1[0:64, :],
                    start=True, stop=True,
                )
            tn = small.tile([128, CH], F32, tag="tn")
            nc.scalar.activation(
                tn[:], nx2.rearrange("p (k two) -> p k two", two=2)[:, :, 0],
                AF.Ln,
            )
            # rx5 = 0.2/||xc||, qneg = -5*rx5 (so den = 1+eps - rx*pm)
            rx5 = persist.tile([128, CH], F32)
            nc.scalar.activation(rx5[:], tn[:], AF.Exp, bias=ln02[:], scale=-0.5)
            qneg = persist.tile([128, CH], F32)
            nc.vector.tensor_scalar_mul(qneg[:], rx5[:], -H_BAND)

            # ---------------- main loop ----------------
            pmall = persist.tile([128, CH], F32)
            aall = persist.tile([128, CH], F32)
            ball = persist.tile([128, CH], F32)
            ssums = persist.tile([128, CH], F32)
            rowbufs = {}

            def chain_and_exp(ks):
                # aa = rx5/den with den = 1+eps - 5*rx5*pm; the exp bias
                # -aa*pm folds to 0.2 - (0.2+0.2*eps)/den exactly (no pm).
                pr = slice(ks[0], ks[-1] + 1)
                nn = len(ks)
                with tc.high_priority():
                    t2 = small.tile([128, nn], F32, tag="t2")
                    nc.vector.tensor_tensor(t2[:], pmall[:, pr], qneg[:, pr], OP.mult)
                    den = small.tile([128, nn], F32, tag="den")
                    nc.vector.tensor_scalar(den[:], t2[:], 1.0 + EPS_MIN, None, OP.add)
                    rec = small.tile([128, nn], F32, tag="rec")
                    nc.vector.reciprocal_approx_fast(rec[:], den[:])
                    nc.vector.tensor_tensor(aall[:, pr], rec[:], rx5[:, pr], OP.mult)
                    nc.vector.tensor_scalar(
                        ball[:, pr], rec[:], -0.2 * (1.0 + EPS_MIN), 0.2,
                        OP.mult, OP.add,
                    )
                for kk in ks:
                    es = espool.tile([128, N], BF16, tag="es")
                    nc.scalar.activation(
                        es[:],
                        rowbufs.pop(kk)[:],
                        AF.Exp,
                        scale=aall[:, kk:kk + 1],
                        bias=ball[:, kk:kk + 1],
                        accum_out=ssums[:, kk:kk + 1],
                    )

            for k in range(CH):
                ck = slice(k * 128, (k + 1) * 128)
                rowbuf = rbpool.tile([128, N], F32, tag="rb")
                rowbufs[k] = rowbuf
                ps0 = mmpool.tile([128, HALF], F32, tag="mm")
                ps1 = mmpool.tile([128, HALF], F32, tag="mm")
                ps = (ps0, ps1)
                for j in range(4):
                    pj = slice(j * 512, (j + 1) * 512)
                    for h in range(2):
                        hp = slice(64 * h, 64 * (h + 1))
                        nc.tensor.matmul(
                            ps[h][:, pj],
                            lhsT=xcen[hp, ck],
                            rhs=yhat[hp, pj],
                            start=True, stop=True,
                        )
                for h in range(2):
                    init = -3.0e38 if h == 0 else pmall[:, k:k + 1]
                    # rowbuf half = copy(ps); pmall[:,k] = max(rowmax, init)
                    nc.vector._custom_dve(
                        TENSOR_MASK_REDUCE,
                        out=rowbuf[:, h * HALF:(h + 1) * HALF],
                        in0=ps[h][:],
                        in1=c3big[:],
                        s0=0.0,
                        s1=init,
                        imm2=1.0,
                        accum_out=pmall[:, k:k + 1],
                    )

                if k >= CH - 2:
                    chain_and_exp((k,))        # shorter tail for last chunks
                elif k % 2 == 1:
                    chain_and_exp((k - 1, k))

            nc.sync.dma_start(out=out_d[:], in_=ssums[:])

    nc.compile()
    return nc


def _get_nc():
    if "nc" not in _NC_CACHE:
        _NC_CACHE["nc"] = build_nc()
    return _NC_CACHE["nc"]


def _make_cst():
    k = np.arange(128)[:, None]
    p = np.arange(128)[None, :]
    identmix = (k % 64 == p % 64).astype(np.float32)
    blockones = (k // 64 == p // 64).astype(np.float32)
    return np.ascontiguousarray(np.concatenate([identmix, blockones], axis=1))


def make_in_maps(X_features, Y_features):
    X = np.ascontiguousarray(np.asarray(X_features, np.float32).reshape(B, C, N))
    Y = np.ascontiguousarray(np.asarray(Y_features, np.float32).reshape(B, C, N))
    cst = _make_cst()
    in_maps = []
    for c in range(8):
        b, h = divmod(c, 2)
        in_maps.append({
            "Xh": np.ascontiguousarray(X[b, :, h * NX:(h + 1) * NX]),
            "Yb": Y[b],
            "Cst": cst,
        })
    return in_maps


def combine(results):
    """results: list of 8 dicts with 'out' [128, CH] = S per row."""
    out = np.empty(B, np.float32)
    for b in range(B):
        tot = 0.0
        for h in range(2):
            s = results[2 * b + h]["out"].astype(np.float64)
            tot += (1.0 / s).sum()
        out[b] = -np.log(tot / N)
    return out


def kernel(X_features, Y_features):
    nc = _get_nc()
    in_maps = make_in_maps(X_features, Y_features)
    res = run_bass_kernel_spmd(nc, in_maps, core_ids=list(range(8)))
    return combine(res.results)


if __name__ == "__main__":
    rng = np.random.default_rng(0)
    X = rng.standard_normal((B, C, 64, 64)).astype(np.float32)
    Y = rng.standard_normal((B, C, 64, 64)).astype(np.float32)
    print(kernel(X_features=X, Y_features=Y))


# revision 24
# speedup vs baseline: 1.2212x; 1.0705x over previous
"""Trainium2 Bass kernel for a contextual loss (cosine-distance softmin loss).

Math (per batch b):
  mu_c      = mean_n Y[b,c,n]
  xc = X-mu, yc = Y-mu                      (centered, [C,N])
  psi[i,j]  = <xc_i, yc_j * ry_j>           (ry = 1/||yc_j||; f32r matmul)
  pm_i      = max_j psi[i,j]
  aa_i      = rx5_i / (1+EPS - 5*rx5_i*pm_i)    (rx5 = 0.2/||xc_i||)
  S_i       = sum_j exp(aa_i*(psi[i,j] - pm_i))
  CX_i      = 1/S_i ;  loss_b = -log(mean_i CX_i)

Sharding: 8 cores = 4 batches x 2 row-halves. Each core gets its
full-batch Y [64,4096] and its half of X's columns [64,2048], returns
S as [128,16] (partition p, chunk k <-> row k*128+p). Host reduces to
the [4] loss.

Layout: Y is loaded PACKED as [128,2048] - partitions 0-63 hold
channels for columns 0-2047, partitions 64-127 for columns 2048-4095,
so all [*,N]-wide preamble work uses all 128 lanes and the K=64 main
matmuls for the two column halves target disjoint PE row-groups.
Cross-half mixing (the mean and per-half channel sums) uses two
constant 0/1 matrices (IdentMix / BlockOnes) fed as an extra input.

Engine budget: the DVE is the bottleneck (32 PSUM->SBUF mask-reduce
copies with fused running row-max). ACT runs only Ln/Exp (squares are
DVE tensor_tensor ops) so a single activation table load suffices.
The exp is ONE ACT op per chunk over [128,4096] with per-row
scale/bias and an accumulated row-sum; per-row constants aa/bb are
computed for PAIRS of chunks ([128,2] ops) to halve small-op count.
"""

import math

import numpy as np

import concourse.bacc as bacc
import concourse.mybir as mybir
from concourse.dve_ops import AFFINE_MUL_REDUCE, TENSOR_MASK_REDUCE
from concourse.bass_utils import run_bass_kernel_spmd
from concourse.mybir import ActivationFunctionType as AF, AluOpType as OP, AxisListType
from concourse.tile import TileContext

F32 = mybir.dt.float32
F32R = mybir.dt.float32r
BF16 = mybir.dt.bfloat16

B, C, N = 4, 64, 4096          # batch, channels, spatial (64*64)
NX = N // 2                    # rows per core (half batch)
CH = NX // 128                 # 16 chunks of 128 rows
HALF = N // 2                  # column half (one packed partition group)
H_BAND = 5.0
EPS_MIN = 1e-3
LN02 = math.log(0.2)

_NC_CACHE = {}


def build_nc():
    nc = bacc.Bacc("TRN2", target_bir_lowering=False, debug=False, num_devices=8)
    x_d = nc.dram_tensor("Xh", [C, NX], F32, kind="ExternalInput")
    y_d = nc.dram_tensor("Yb", [C, N], F32R, kind="ExternalInput")
    cst_d = nc.dram_tensor("Cst", [128, 256], F32R, kind="ExternalInput")
    out_d = nc.dram_tensor("out", [128, CH], F32, kind="ExternalOutput")

    with TileContext(nc) as tc:
        with (
            tc.tile_pool(name="persist", bufs=1) as persist,
            tc.tile_pool(name="mm", bufs=2, space="PSUM") as mmpool,
            tc.tile_pool(name="rb", bufs=6) as rbpool,
            tc.tile_pool(name="ry", bufs=2) as rypool,
            tc.tile_pool(name="es", bufs=2) as espool,
            tc.tile_pool(name="small", bufs=4) as small,
        ):
            # ---------------- load inputs (packed Y first, then X) ------------
            cst = persist.tile([128, 256], F32R)
            nc.sync.dma_start(out=cst[:], in_=cst_d[:])
            identmix = cst[:, 0:128]          # [k,p]=1 iff k%64==p%64
            blockones = cst[:, 128:256]       # [k,p]=1 iff k//64==p//64

            yp = persist.tile([128, HALF], F32R)   # packed Y
            for q in range(4):
                h, qq = divmod(q, 2)
                nc.sync.dma_start(
                    out=yp[64 * h:64 * (h + 1), qq * 1024:(qq + 1) * 1024],
                    in_=y_d[:, h * HALF + qq * 1024:h * HALF + (qq + 1) * 1024],
                )
            x_sb = persist.tile([C, NX], F32)
            nc.sync.dma_start(out=x_sb[:], in_=x_d[:])

            ones_f = persist.tile([128, 64], F32)
            nc.vector.memset(ones_f[:], 1.0)
            ones1 = persist.tile([128, 2], F32R)
            nc.vector.tensor_copy(ones1[:], ones_f[:, 0:2])
            c3big = persist.tile([128, 1], F32)
            nc.vector.memset(c3big[:], 1.0e9)
            ln02 = persist.tile([128, 1], F32)
            nc.vector.memset(ln02[:], LN02)

            # one PSUM tile shared by all small preamble matmuls (disjoint
            # column ranges), so the 2-buffer PSUM pool never deadlocks on
            # the long-lived ny2 tile.
            aux = mmpool.tile([128, HALF], F32, tag="mm")
            ps_mu = aux[:, 0:2]
            ps_w = aux[:, 2:4]
            nx2 = aux[:, 64:64 + 2 * CH]

            # squares + mean partials of RAW packed Y overlap the DMA
            ysq = persist.tile([128, HALF], F32R)
            muparts = small.tile([128, 2], F32, tag="muparts")
            ny2 = mmpool.tile([128, HALF], F32, tag="mm")
            for q in range(2):
                qs = slice(q * 1024, (q + 1) * 1024)
                ypq = yp[:, qs].bitcast(F32)
                nc.vector.reduce_sum(
                    out=muparts[:, q:q + 1], in_=ypq, axis=AxisListType.X
                )
                nc.vector.tensor_tensor(ysq[:, qs], ypq, ypq, OP.mult)
                # ny2 partial: BlockOnes @ Y^2  (finished by the -2mu@Y MM below)
                for j in range(2):
                    pj = slice(q * 1024 + j * 512, q * 1024 + (j + 1) * 512)
                    nc.tensor.matmul(
                        ny2[:, pj], lhsT=blockones, rhs=ysq[:, pj],
                        start=True, stop=False,
                    )

            musum = small.tile([128, 1], F32, tag="musum")
            nc.vector.reduce_sum(
                out=musum[:], in_=muparts[:], axis=AxisListType.X
            )
            musr = small.tile([128, 2], F32R, tag="musr")
            nc.vector.tensor_scalar(musr[:], ones_f[:, 0:2], musum[:], None, OP.mult)
            # full-row mean on every partition: mu[p] = sum of both halves / N
            nc.tensor.matmul(
                ps_mu, lhsT=identmix, rhs=musr[:], start=True, stop=True
            )
            mu = small.tile([128, 1], F32, tag="mu")
            nc.vector.tensor_scalar_mul(mu[:], ps_mu[:, 0:1], 1.0 / N)
            muneg = small.tile([128, 1], F32, tag="muneg")
            nc.vector.tensor_scalar_mul(muneg[:], mu[:], -1.0)
            mu2neg = small.tile([128, 1], F32, tag="mu2neg")
            nc.vector.tensor_scalar_mul(mu2neg[:], mu[:], -2.0)

            # lhsT for the -2*mu@Y correction and for w = |mu|^2 broadcast
            lhs_mu = persist.tile([128, 128], F32R)
            nc.vector.tensor_scalar(
                lhs_mu[:], blockones.bitcast(F32), mu2neg[:], None, OP.mult
            )
            musq = small.tile([128, 1], F32, tag="musq")
            nc.vector.tensor_tensor(musq[:], mu[:], mu[:], OP.mult)
            lhs_w = persist.tile([128, 128], F32R)
            nc.vector.tensor_scalar(
                lhs_w[:], blockones.bitcast(F32), musq[:], None, OP.mult
            )
            nc.tensor.matmul(
                ps_w, lhsT=lhs_w[:], rhs=ones1[:], start=True, stop=True
            )
            w_sb = small.tile([128, 1], F32, tag="w")
            nc.vector.tensor_copy(w_sb[:], ps_w[:, 0:1])

            # ---------------- Y norms + yhat (packed halves) ------------------
            # finish ny2 with the -2mu@Y MMs; ry = exp(-0.5*ln(ny2+w));
            # yhat = (Y - mu)*ry in one fused DVE op. Both Lns are emitted
            # before both Exps so the activation table set switches ONCE.
            yhat = persist.tile([128, HALF], F32R)
            for j in range(4):
                pj = slice(j * 512, (j + 1) * 512)
                nc.tensor.matmul(
                    ny2[:, pj], lhsT=lhs_mu[:], rhs=yp[:, pj],
                    start=False, stop=True,
                )
            tln0 = rypool.tile([128, 1024], F32, tag="tln")
            nc.scalar.activation(tln0[:], ny2[:, 0:1024], AF.Ln, bias=w_sb[:])
            tln1 = rypool.tile([128, 1024], F32, tag="tln")
            nc.scalar.activation(tln1[:], ny2[:, 1024:2048], AF.Ln, bias=w_sb[:])
            for v, tln in ((0, tln0), (1, tln1)):
                vs = slice(v * 1024, (v + 1) * 1024)
                ry = rypool.tile([128, 1024], F32, tag="ry")
                nc.scalar.activation(ry[:], tln[:], AF.Exp, scale=-0.5)
                nc.vector._custom_dve(
                    AFFINE_MUL_REDUCE,
                    out=yhat[:, vs],
                    in0=yp[:, vs].bitcast(F32),
                    in1=ry[:],
                    s0=1.0,
                    s1=muneg[:],
                )

            # ---------------- X side: center, norms, rx5 ----------------------
            # X is centered on partitions 0-63 and duplicated to 64-127 by an
            # SBUF->SBUF DMA (PE row-tiling needs both row-groups).
            xcen = persist.tile([128, NX], F32R)
            nc.vector.tensor_scalar(xcen[0:64, :], x_sb[:], mu[0:64, :], None, OP.subtract)
            nc.sync.dma_start(out=xcen[64:128, :], in_=xcen[0:64, :])
            xsq = persist.tile([C, NX], F32R)
            nc.vector.tensor_tensor(
                xsq[:], xcen[0:64, :].bitcast(F32), xcen[0:64, :].bitcast(F32),
                OP.mult,
            )
            for k in range(CH):
                nc.tensor.matmul(
                    nx2[:, 2 * k:2 * k + 2],
                    lhsT=xsq[:, k * 128:(k + 1) * 128],
                    rhs=ones1[0:64, :],
                    start=True, stop=True,
                )
            tn = small.tile([128, CH], F32, tag="tn")
            nc.scalar.activation(
                tn[:], nx2.rearrange("p (k two) -> p k two", two=2)[:, :, 0],
                AF.Ln,
            )
            # rx5 = 0.2/||xc||, qneg = -5*rx5 (so den = 1+eps - rx*pm)
            rx5 = persist.tile([128, CH], F32)
            nc.scalar.activation(rx5[:], tn[:], AF.Exp, bias=ln02[:], scale=-0.5)
            qneg = persist.tile([128, CH], F32)
            nc.vector.tensor_scalar_mul(qneg[:], rx5[:], -H_BAND)

            # ---------------- main loop ----------------
            pmall = persist.tile([128, CH], F32)
            aall = persist.tile([128, CH], F32)
            ball = persist.tile([128, CH], F32)
            ssums = persist.tile([128, CH], F32)
            rowbufs = {}

            def chain_and_exp(ks):
                # aa = rx5/den with den = 1+eps - 5*rx5*pm; the exp bias
                # -aa*pm folds to 0.2 - (0.2+0.2*eps)/den exactly (no pm).
                # All small ops run on the otherwise-idle GPSIMD except the
                # reciprocal (DVE-only custom op): every sem-incrementing
                # DVE op costs a full mask slot in the scheduled stream.
                pr = slice(ks[0], ks[-1] + 1)
                nn = len(ks)
                t2 = small.tile([128, nn], F32, tag="t2")
                nc.gpsimd.tensor_tensor(t2[:], pmall[:, pr], qneg[:, pr], OP.mult)
                den = small.tile([128, nn], F32, tag="den")
                nc.gpsimd.tensor_scalar(den[:], t2[:], 1.0 + EPS_MIN, None, OP.add)
                rec = small.tile([128, nn], F32, tag="rec")
                with tc.high_priority():
                    nc.vector.reciprocal_approx_fast(rec[:], den[:])
                nc.gpsimd.tensor_tensor(aall[:, pr], rec[:], rx5[:, pr], OP.mult)
                nc.gpsimd.tensor_scalar(
                    ball[:, pr], rec[:], -0.2 * (1.0 + EPS_MIN), 0.2,
                    OP.mult, OP.add,
                )
                for kk in ks:
                    es = espool.tile([128, N], BF16, tag="es")
                    nc.scalar.activation(
                        es[:],
                        rowbufs.pop(kk)[:],
                        AF.Exp,
                        scale=aall[:, kk:kk + 1],
                        bias=ball[:, kk:kk + 1],
                        accum_out=ssums[:, kk:kk + 1],
                    )

            for k in range(CH):
                ck = slice(k * 128, (k + 1) * 128)
                rowbuf = rbpool.tile([128, N], F32, tag="rb")
                rowbufs[k] = rowbuf
                ps0 = mmpool.tile([128, HALF], F32, tag="mm")
                ps1 = mmpool.tile([128, HALF], F32, tag="mm")
                ps = (ps0, ps1)
                for j in range(4):
                    pj = slice(j * 512, (j + 1) * 512)
                    for h in range(2):
                        hp = slice(64 * h, 64 * (h + 1))
                        nc.tensor.matmul(
                            ps[h][:, pj],
                            lhsT=xcen[hp, ck],
                            rhs=yhat[hp, pj],
                            start=True, stop=True,
                        )
                for h in range(2):
                    init = -3.0e38 if h == 0 else pmall[:, k:k + 1]
                    # rowbuf half = copy(ps); pmall[:,k] = max(rowmax, init)
                    nc.vector._custom_dve(
                        TENSOR_MASK_REDUCE,
                        out=rowbuf[:, h * HALF:(h + 1) * HALF],
                        in0=ps[h][:],
                        in1=c3big[:],
                        s0=0.0,
                        s1=init,
                        imm2=1.0,
                        accum_out=pmall[:, k:k + 1],
                    )

                if k >= CH - 2:
                    chain_and_exp((k,))        # shorter tail for last chunks
                elif k % 2 == 1:
                    chain_and_exp((k - 1, k))

            nc.sync.dma_start(out=out_d[:], in_=ssums[:])

    nc.compile()
    return nc


def _get_nc():
    if "nc" not in _NC_CACHE:
        _NC_CACHE["nc"] = build_nc()
    return _NC_CACHE["nc"]


def _make_cst():
    k = np.arange(128)[:, None]
    p = np.arange(128)[None, :]
    identmix = (k % 64 == p % 64).astype(np.float32)
    blockones = (k // 64 == p // 64).astype(np.float32)
    return np.ascontiguousarray(np.concatenate([identmix, blockones], axis=1))


def make_in_maps(X_features, Y_features):
    X = np.ascontiguousarray(np.asarray(X_features, np.float32).reshape(B, C, N))
    Y = np.ascontiguousarray(np.asarray(Y_features, np.float32).reshape(B, C, N))
    cst = _make_cst()
    in_maps = []
    for c in range(8):
        b, h = divmod(c, 2)
        in_maps.append({
            "Xh": np.ascontiguousarray(X[b, :, h * NX:(h + 1) * NX]),
            "Yb": Y[b],
            "Cst": cst,
        })
    return in_maps


def combine(results):
    """results: list of 8 dicts with 'out' [128, CH] = S per row."""
    out = np.empty(B, np.float32)
    for b in range(B):
        tot = 0.0
        for h in range(2):
            s = results[2 * b + h]["out"].astype(np.float64)
            tot += (1.0 / s).sum()
        out[b] = -np.log(tot / N)
    return out


def kernel(X_features, Y_features):
    nc = _get_nc()
    in_maps = make_in_maps(X_features, Y_features)
    res = run_bass_kernel_spmd(nc, in_maps, core_ids=list(range(8)))
    return combine(res.results)


if __name__ == "__main__":
    rng = np.random.default_rng(0)
    X = rng.standard_normal((B, C, 64, 64)).astype(np.float32)
    Y = rng.standard_normal((B, C, 64, 64)).astype(np.float32)
    print(kernel(X_features=X, Y_features=Y))
